# revision 53
# baseline (speedup 1.0000x reference)
"""GTLayer (graph transformer layer) distributed Bass kernel for 8 TRN2 cores.

The axon tunnel moves ~47 MB/s host->device (shared across all 8 devices)
and ~34 MB/s back, so the design minimizes host<->device bytes: each core
receives only its NODE SHARD of q/k/v (all fp8-e3m4; Wk/Wv fp8 x4-scaled to
dodge subnormals, compensated via Wq and Wo; q's residual-path quantization
error is repaired exactly on the host via dq = q - fp8(q) times the
BN1*BN2 scales the device ships back — only the nonlinear FFN-path error
remains), the per-edge head
biases eb = ef@We.T+be (computed on host, 8 fp8 cols instead of 64 f32),
per-edge src pair-indices (int16, gpsimd-wrapped), parity + dstrel (u8), and
1/8 of the shared weight block (AllGathered on device) — ~3.0 MB/core vs
29 MB for shipping per-edge gathered k/v rows.

On device each core projects K'=k@Wk.T, V'=v@Wv.T for its 5000 nodes, packs
rows [K'|V'] (256 bf16) into a DRAM bounce buffer, AllGathers the full
40000-row table over NeuronLink, then uses gpsimd dma_gather (paged-KV
machinery, mlp ucode library; ~1us per gathered row) to fetch per-edge
pair-rows (2 nodes per 1KB row since indices are int16) in <=1024-edge
chunks; a parity blend selects the right node.  Padded slots index row 0 —
an all-padding chunk with -1 "ignored" indices would emit zero DMA
descriptors and never fire its completion semaphore (deadlock).  The rest is
the proven pipeline: per-group q projection + one-hot gather, per-edge dots
(DVE), exp (ACT), segment softmax-sum + weighted aggregation via one-hot
matmuls into PSUM, then Wo + residual + BN (global stats via AllReduce) +
FFN + BN2 partial stats (finished host-side).  Donated output zero-buffers
are created on device (jitted, pre-dispatched at kernel() entry) and output
shards are fetched with concurrent threads.
"""

import json
from contextlib import ExitStack
import numpy as np
import ml_dtypes

import concourse.bass as bass
import concourse.mybir as mybir
import concourse.tile as tile
from concourse import library_config
from concourse.library_overlay import lower_extended_insts
from concourse.bass_utils import run_bass_kernel_spmd

bf16 = ml_dtypes.bfloat16

# problem constants (hardcoded per contract)
N, E, IN, H, D, ED = 40000, 640000, 128, 8, 16, 64
C = H * D            # 128
NCORE = 8
NSH = N // NCORE     # 5000 nodes per core
NG = 40              # node groups of <=128 per core (39*128+8)
EPS = 1e-5
CHUNK_T = 8          # tiles per dma_gather call (1024 idxs fits the 16KB carveout)

f32 = mybir.dt.float32
bft = mybir.dt.bfloat16
i16 = mybir.dt.int16
f8 = mybir.dt.float8e3
u8 = mybir.dt.uint8
e3m4 = ml_dtypes.float8_e3m4
WS = 4.0   # fp8 weight scale (Wk,Wv x4 dodges subnormals; undone via Wq, Wo)


def _split_multiwaits_json(bir: bytes) -> bytes:
    """This walrus build allows only ONE sem wait per instruction; Tile emits
    multi-waits.  Split extras onto NoOps inserted before, same engine."""
    b = json.loads(bir)
    ctr = [0]
    changed = False
    for f in b.get("functions", []):
        for blk in f.get("blocks", []):
            insts = blk.get("instructions")
            if not insts:
                continue
            out = []
            for i in insts:
                si = i.get("sync_info")
                waits = (si or {}).get("on_wait") or []
                if len(waits) > 1:
                    changed = True
                    for w in waits[:-1]:
                        ctr[0] += 1
                        out.append({
                            "debug": i.get("debug", 0), "engine": i["engine"],
                            "ins": [], "name": f"I-wsplit-{ctr[0]}",
                            "opcode": "NoOp", "outs": [],
                            "text_hint": "wsplit",
                            "sync_info": {"on_update": [], "on_wait": [w]},
                        })
                    si["on_wait"] = [waits[-1]]
                out.append(i)
            blk["instructions"] = out
    return json.dumps(b).encode() if changed else bir


class _BassW(bass.Bass):
    def to_json_bytes(self) -> bytes:
        return _split_multiwaits_json(super().to_json_bytes())


_DT_SIZE = {f32: 4, bft: 2, i16: 2, f8: 1, u8: 1}


# shared weight block: identical on every core, so each core ships only a
# 1/8 byte-slice and the device AllGathers the full block (saves 7/8 of the
# replicated-weight tunnel bytes)
_WSPEC = [
    ("iota", bft, [128, 128]),
    ("WqT", bft, [IN, C]), ("WoT", bft, [C, C]),
    ("W1Ta", bft, [C, C]), ("W1Tb", bft, [C, C]),
    ("W2Ta", bft, [C, C]), ("W2Tb", bft, [C, C]),
    ("WkT", f8, [IN, C]), ("WvT", f8, [IN, C]),
    ("b1a", f32, [128, 1]), ("b1b", f32, [128, 1]), ("b2", f32, [128, 1]),
    ("g1", f32, [128, 1]), ("bt1", f32, [128, 1]),
    ("g2", f32, [128, 1]), ("bt2", f32, [128, 1]),
]
WBYTES = sum(int(np.prod(sh)) * {f32: 4, bft: 2, f8: 1}[dt]
             for _, dt, sh in _WSPEC)          # 265728
WSLICE = WBYTES // NCORE


def _input_spec(T_all):
    """Single packed uint8 blob layout: 2-byte sections first, then 1-byte
    (keeps every section aligned to its dtype size)."""
    S16 = (128 * T_all) // 16
    return [
        ("idxw", i16, [16, S16]),
        ("eb8", f8, [128, 8 * T_all]),
        ("dstrel", u8, [128, T_all]),   # 0..127; 255 = padded lane
        ("par", u8, [128, T_all]),
        ("wsh", u8, [1, WSLICE]),       # this core's slice of _WSPEC bytes
    ]


def _build_program(tiles_per_group: tuple[tuple[int, ...], ...]):
    """tiles_per_group: per-core tuple of NG tile counts.  All cores run the
    same SPMD program, so loop bounds use the per-group MAX across cores;
    cores with fewer edges in a group just process padded tiles (padded
    slots gather row 0 — finite; dstrel 255 -> one-hot zero -> zero
    contribution to the aggregation)."""
    tg = tuple(max(tiles_per_group[m][g] for m in range(NCORE))
               for g in range(NG))
    T_all = sum(tg)
    S16 = (128 * T_all) // 16
    nc = _BassW()
    spec = _input_spec(T_all)
    total = sum(int(np.prod(sh)) * _DT_SIZE[dt] for _, dt, sh in spec)
    # q/k/v shards don't depend on the edge sort, so they ship as their own
    # tensor whose upload starts at kernel() entry, before argsort
    SQ = IN * NSH
    aqkv = nc.dram_tensor("aqkv", [3 * SQ], mybir.dt.uint8, kind="ExternalInput")
    blob = nc.dram_tensor("blob", [total], mybir.dt.uint8, kind="ExternalInput")
    dins = {}
    off = 0
    for name, dt, sh in spec:
        nbytes = int(np.prod(sh)) * _DT_SIZE[dt]
        dins[name] = blob[off:off + nbytes].bitcast(dt).rearrange(
            "(p f) -> p f", p=sh[0])
        off += nbytes
    for i, name in enumerate(("kT", "vT", "qT8")):
        dins[name] = aqkv[i * SQ:(i + 1) * SQ].bitcast(f8).rearrange(
            "(p f) -> p f", p=IN)
    dout = nc.dram_tensor("out", [C, NSH], bft, kind="ExternalOutput")
    # BN2 is finished on host during unshard: device emits partial sums
    # (sum y, sum y^2 per channel); host combines the 8 shards and applies
    # the per-channel affine while gathering the full output.
    dstat = nc.dram_tensor("stats", [128, 3], f32, kind="ExternalOutput")

    CH = 500  # phase-2 node chunk
    NCH = NSH // CH

    with tile.TileContext(nc) as tc:
        with (
            tc.tile_pool(name="const", bufs=1) as cpool,
            tc.tile_pool(name="wts", bufs=1) as wpool,
            tc.tile_pool(name="edge", bufs=3) as epool,
            tc.tile_pool(name="big", bufs=1) as bpool,
            tc.tile_pool(name="dram", bufs=1, space="DRAM") as dpool,
        ):
            # ---- weights: AllGather the sharded block, then load tiles ----
            wout = nc.dram_tensor("wgat", [WBYTES], mybir.dt.uint8,
                                  kind="Internal")
            # collectives can't read IO tensors: bounce the slice first
            wgin = nc.dram_tensor("wgin", [WSLICE], mybir.dt.uint8,
                                  kind="Internal")
            nc.sync.dma_start(out=wgin[:].rearrange("(a b) -> a b", a=1),
                              in_=dins["wsh"][:])
            nc.gpsimd.collective_compute(
                "AllGather", mybir.AluOpType.bypass,
                replica_groups=[list(range(NCORE))],
                ins=[wgin[:].opt()], outs=[wout[:].opt()])
            wins = {}
            woff = 0
            for name, dt, sh in _WSPEC:
                nbytes = int(np.prod(sh)) * _DT_SIZE[dt]
                wins[name] = wout[woff:woff + nbytes].bitcast(dt).rearrange(
                    "(p f) -> p f", p=sh[0])
                woff += nbytes
            iota_t = cpool.tile([128, 128], bft)
            nc.sync.dma_start(out=iota_t[:], in_=wins["iota"][:])
            w = {}
            for nm in ("WqT", "WoT", "W1Ta", "W1Tb", "W2Ta", "W2Tb"):
                w[nm] = wpool.tile([C, C], bft, name=nm, tag=nm)
                nc.sync.dma_start(out=w[nm][:], in_=wins[nm][:])
            for nm in ("WkT", "WvT"):
                w[nm] = wpool.tile([C, C], f8, name=nm, tag=nm)
                nc.sync.dma_start(out=w[nm][:], in_=wins[nm][:])
            vec = {}
            for nm in ("b1a", "b1b", "b2", "g1", "bt1", "g2", "bt2"):
                vec[nm] = wpool.tile([128, 1], f32, name=nm, tag=nm)
                nc.sync.dma_start(out=vec[nm][:], in_=wins[nm][:])
            qT8_t = bpool.tile([IN, NSH], f8)
            nc.sync.dma_start(out=qT8_t[:], in_=dins["qT8"][:])
            qT_t = bpool.tile([IN, NSH], bft)
            nc.vector.tensor_copy(qT_t[:], qT8_t[:])

            # identity for PE transposes; iota column for one-hot building
            ident = cpool.tile([128, 128], bft)
            iota_col = cpool.tile([128, 1], mybir.dt.int32)
            nc.gpsimd.iota(iota_col[:], [[0, 1]], channel_multiplier=1)
            # gpsimd: switch to the mlp ucode library (dma_gather) AFTER iota
            nc.gpsimd.load_library(library_config.mlp)
            iota_col_f = cpool.tile([128, 1], bft)
            nc.vector.tensor_copy(iota_col_f[:], iota_col[:])
            nc.vector.tensor_tensor(
                out=ident[:], in0=iota_col_f[:].to_broadcast([128, 128]),
                in1=iota_t[:], op=mybir.AluOpType.is_equal)

            # edge metadata resident in SBUF (eb ships fp8, upconvert once)
            eb8_sb = bpool.tile([128, 8 * T_all], f8)
            nc.sync.dma_start(out=eb8_sb[:], in_=dins["eb8"][:])
            eb_sb = bpool.tile([128, 8 * T_all], bft)
            nc.vector.tensor_copy(eb_sb[:], eb8_sb[:])
            dr8 = bpool.tile([128, T_all], u8)
            nc.sync.dma_start(out=dr8[:], in_=dins["dstrel"][:])
            dr_all = bpool.tile([128, T_all], bft)
            nc.vector.tensor_copy(dr_all[:], dr8[:])
            par8 = bpool.tile([128, T_all], u8)
            nc.sync.dma_start(out=par8[:], in_=dins["par"][:])
            par_all = bpool.tile([128, T_all], bft)
            nc.vector.tensor_copy(par_all[:], par8[:])
            idx_sb = bpool.tile([128, S16], i16)
            nc.sync.dma_start(out=idx_sb[0:16, :], in_=dins["idxw"][:])
            # replicate the wrapped indices to all 8 gpsimd 16-partition blocks
            nc.sync.dma_start(out=idx_sb[16:32, :], in_=idx_sb[0:16, :])
            nc.sync.dma_start(out=idx_sb[32:64, :], in_=idx_sb[0:32, :])
            nc.sync.dma_start(out=idx_sb[64:128, :], in_=idx_sb[0:64, :])

            # ---- phase 0: project K' V' for own shard, AllGather tables ----
            ph0 = ExitStack()
            kvpool = ph0.enter_context(tc.tile_pool(name="kvin", bufs=1))
            pppool = ph0.enter_context(tc.tile_pool(name="p0ps", bufs=2, space="PSUM"))
            stpool = ph0.enter_context(tc.tile_pool(name="p0st", bufs=2))
            kT_t = kvpool.tile([IN, NSH], f8)
            nc.sync.dma_start(out=kT_t[:], in_=dins["kT"][:])
            vT_t = kvpool.tile([IN, NSH], f8)
            nc.sync.dma_start(out=vT_t[:], in_=dins["vT"][:])
            bounce_in = dpool.tile([NSH, 2 * C], bft)      # [K'|V'] per node
            # pair layout: row i = [K'(2i)|V'(2i)|K'(2i+1)|V'(2i+1)], 512 bf16
            bounce_out = dpool.tile([N // 2, 4 * C], bft)
            for g in range(NG):
                n_lo = g * 128
                n_hi = min(NSH - n_lo, 128)
                ps = pppool.tile([128, 2 * C], f32, tag="p0")
                nc.tensor.matmul(ps[:n_hi, 0:C], lhsT=kT_t[:, n_lo:n_lo + n_hi],
                                 rhs=w["WkT"][:], start=True, stop=True)
                nc.tensor.matmul(ps[:n_hi, C:2 * C], lhsT=vT_t[:, n_lo:n_lo + n_hi],
                                 rhs=w["WvT"][:], start=True, stop=True)
                st = stpool.tile([128, 2 * C], bft, tag="p0s")
                nc.vector.tensor_copy(st[:n_hi, :], ps[:n_hi, :])
                nc.sync.dma_start(out=bounce_in[n_lo:n_lo + n_hi, :],
                                  in_=st[:n_hi, :])
            nc.gpsimd.collective_compute(
                "AllGather", mybir.AluOpType.bypass,
                replica_groups=[list(range(NCORE))],
                ins=[bounce_in.opt()], outs=[bounce_out.opt()])
            ph0.close()
            pair_tbl = bounce_out[:]

            # normalized aggregation output, channel-major, bf16
            aggT_sb = bpool.tile([C, NSH], bft)

            # ---- phase 1: per group ----
            ph1 = ExitStack()
            gpool = ph1.enter_context(tc.tile_pool(name="grpin", bufs=2))
            pspool = ph1.enter_context(tc.tile_pool(name="eps", bufs=2, space="PSUM"))
            qgpool = ph1.enter_context(tc.tile_pool(name="qps", bufs=2, space="PSUM"))
            aggpool = ph1.enter_context(tc.tile_pool(name="agg", bufs=2, space="PSUM"))
            nregs = {}

            def nreg(v):
                if v not in nregs:
                    nregs[v] = nc.gpsimd.to_reg(v)
                return nregs[v]

            toff = 0
            for g in range(NG):
                n_lo = g * 128
                n_hi = min(NSH - n_lo, 128)
                # per-group q projection: qp_g [n_hi, C] (node-major)
                qg_ps = qgpool.tile([128, C], f32, tag="grp")
                nc.tensor.matmul(qg_ps[:n_hi, :], lhsT=qT_t[:, n_lo:n_lo + n_hi],
                                 rhs=w["WqT"][:], start=True, stop=True)
                qg_sb = epool.tile([128, C], bft, tag="qgs")
                nc.vector.tensor_copy(qg_sb[:n_hi, :], qg_ps[:n_hi, :])

                agg_ps = aggpool.tile([128, C + H], f32)
                T = tg[g]
                # gather [K|V|K|V] pair rows for all edges of this group, in
                # chunks of <= CHUNK_T tiles (dynamic-DMA carveout limit)
                pbuf = gpool.tile([128, 512 * T], bft, tag="pbuf")
                # padded lanes gather row 0 (never -1: an all-padding chunk
                # would emit zero descriptors and deadlock); memset is
                # belt-and-suspenders so no lane can ever hold stale NaN bits
                nc.vector.memset(pbuf[:], 0.0)
                for c0 in range(0, T, CHUNK_T):
                    if DBG.get("nogather") or g >= DBG.get("gather_groups", NG):
                        break
                    ct = min(CHUNK_T, T - c0)
                    j0 = (toff + c0) * 8  # 128/16 idx cols per tile
                    nc.gpsimd.dma_gather(
                        out_ap=pbuf[:, 512 * c0:512 * (c0 + ct)].rearrange(
                            "p (t e) -> p t e", e=512),
                        in_ap=pair_tbl,
                        idxs_ap=idx_sb[:, j0:j0 + 8 * ct],
                        num_idxs=128 * ct, num_idxs_reg=nreg(128 * ct),
                        elem_size=512)
                for t in range(T):
                    pair = pbuf[:, 512 * t:512 * (t + 1)]
                    dr = dr_all[:, toff + t: toff + t + 1]
                    parc = par_all[:, toff + t: toff + t + 1]
                    ebt = eb_sb[:, 8 * (toff + t): 8 * (toff + t) + 8]

                    # parity blend: x = even + (odd - even) * par
                    kvp = epool.tile([128, 2 * C], bft, tag="kvp")
                    kp = kvp[:, 0:C]
                    vp = kvp[:, C:2 * C]
                    dif = epool.tile([128, 2 * C], bft, tag="dif")
                    nc.vector.tensor_tensor(
                        out=dif[:], in0=pair[:, 256:512], in1=pair[:, 0:256],
                        op=mybir.AluOpType.subtract)
                    nc.vector.tensor_tensor(
                        out=dif[:], in0=dif[:],
                        in1=parc.to_broadcast([128, 2 * C]),
                        op=mybir.AluOpType.mult)
                    nc.vector.tensor_tensor(
                        out=kvp[:], in0=pair[:, 0:256], in1=dif[:],
                        op=mybir.AluOpType.add)

                    # one-hot [e, n] and its transpose [n, e]
                    oh = epool.tile([128, 128], bft, tag="oh")
                    nc.vector.tensor_tensor(
                        out=oh[:], in0=dr.to_broadcast([128, 128]),
                        in1=iota_t[:], op=mybir.AluOpType.is_equal)
                    ohT_ps = qgpool.tile([128, 128], bft, tag="ohT")
                    nc.tensor.transpose(ohT_ps[:], oh[:], ident[:])
                    ohT = epool.tile([128, 128], bft, tag="ohTs")
                    nc.vector.tensor_copy(ohT[:], ohT_ps[:])

                    # per-edge q via one-hot gather of qp_g
                    qp_ps = pspool.tile([128, C], f32, tag="qp")
                    nc.tensor.matmul(qp_ps[:], lhsT=ohT[:n_hi, :],
                                     rhs=qg_sb[:n_hi, :], start=True, stop=True)

                    # scores
                    prod = epool.tile([128, C], bft, tag="prod")
                    nc.vector.tensor_tensor(out=prod[:], in0=kp, in1=qp_ps[:],
                                            op=mybir.AluOpType.mult)
                    s0 = epool.tile([128, H], f32, tag="s0")
                    nc.vector.tensor_reduce(
                        out=s0[:], in_=prod[:].rearrange("p (h d) -> p h d", h=H),
                        axis=mybir.AxisListType.X, op=mybir.AluOpType.add)
                    sc = epool.tile([128, H], f32, tag="sc")
                    nc.vector.tensor_tensor(out=sc[:], in0=s0[:], in1=ebt,
                                            op=mybir.AluOpType.add)
                    # rhs tile [Vw | ex];  ex = exp(sc)
                    rhs = epool.tile([128, C + H], bft, tag="rhs")
                    ex = rhs[:, C:C + H]
                    nc.scalar.activation(ex, sc[:], mybir.ActivationFunctionType.Exp)
                    nc.vector.tensor_tensor(
                        out=rhs[:, 0:C].rearrange("p (h d) -> p h d", h=H),
                        in0=vp.rearrange("p (h d) -> p h d", h=H),
                        in1=ex.to_broadcast([128, H, D]),
                        op=mybir.AluOpType.mult)
                    nc.tensor.matmul(agg_ps[:], lhsT=oh[:], rhs=rhs[:],
                                     start=(t == 0), stop=(t == T - 1))
                toff += T

                # normalize by denominator and transpose to channel-major
                rec = epool.tile([128, H], f32, tag="rec")
                nc.vector.reciprocal(rec[:], agg_ps[:, C:C + H])
                aggn = epool.tile([128, C], bft, tag="aggn")
                nc.vector.tensor_tensor(
                    out=aggn[:].rearrange("p (h d) -> p h d", h=H),
                    in0=agg_ps[:, 0:C].rearrange("p (h d) -> p h d", h=H),
                    in1=rec[:].to_broadcast([128, H, D]),
                    op=mybir.AluOpType.mult)
                aggnT_ps = qgpool.tile([128, 128], bft, tag="grp")
                nc.tensor.transpose(aggnT_ps[:], aggn[:], ident[:])
                nc.vector.tensor_copy(aggT_sb[:, n_lo:n_lo + n_hi],
                                      aggnT_ps[:, 0:n_hi])

            ph1.close()
            # ---- phase 2: channel-major dense ----
            p2ctx = ExitStack()
            p2pool = p2ctx.enter_context(tc.tile_pool(name="ph2ps", bufs=2, space="PSUM"))
            rst = bpool.tile([C, NSH], f32)
            for ci in range(NCH):
                s0_ = ci * CH
                ps = p2pool.tile([128, CH], f32, tag="wo")
                nc.tensor.matmul(ps[:], lhsT=w["WoT"][:],
                                 rhs=aggT_sb[:, s0_:s0_ + CH], start=True, stop=True)
                nc.vector.tensor_tensor(out=rst[:, s0_:s0_ + CH], in0=ps[:],
                                        in1=qT_t[:, s0_:s0_ + CH],
                                        op=mybir.AluOpType.add)

            def bn_layer(x_sb, gv, btv, suffix):
                # global mean/var across all N nodes (AllReduce of sum/sumsq)
                st = bpool.tile([128, 2], f32, tag=f"st{suffix}")
                nc.vector.tensor_reduce(out=st[:, 0:1], in_=x_sb[:],
                                        axis=mybir.AxisListType.X,
                                        op=mybir.AluOpType.add)
                sq = bpool.tile([C, NSH], bft, tag="sqscratch")
                nc.scalar.activation(sq[:], x_sb[:],
                                     mybir.ActivationFunctionType.Square,
                                     accum_out=st[:, 1:2])
                bounce_in = dpool.tile([128, 2], f32, tag=f"bi{suffix}")
                bounce_out = dpool.tile([128, 2], f32, tag=f"bo{suffix}")
                nc.gpsimd.dma_start(out=bounce_in[:], in_=st[:])
                nc.gpsimd.collective_compute(
                    "AllReduce", mybir.AluOpType.add,
                    replica_groups=[list(range(NCORE))],
                    ins=[bounce_in.opt()], outs=[bounce_out.opt()])
                stg = bpool.tile([128, 2], f32, tag=f"stg{suffix}")
                nc.sync.dma_start(out=stg[:], in_=bounce_out[:])
                mean = bpool.tile([128, 1], f32, tag=f"mean{suffix}")
                nc.vector.tensor_scalar_mul(mean[:], stg[:, 0:1], 1.0 / N)
                msq = bpool.tile([128, 1], f32, tag=f"msq{suffix}")
                nc.scalar.activation(msq[:], mean[:],
                                     mybir.ActivationFunctionType.Square)
                var = bpool.tile([128, 1], f32, tag=f"var{suffix}")
                nc.vector.tensor_scalar_mul(var[:], stg[:, 1:2], 1.0 / N)
                nc.vector.tensor_tensor(out=var[:], in0=var[:], in1=msq[:],
                                        op=mybir.AluOpType.subtract)
                nc.vector.tensor_scalar_add(var[:], var[:], float(EPS))
                sd = bpool.tile([128, 1], f32, tag=f"sd{suffix}")
                nc.scalar.activation(sd[:], var[:],
                                     mybir.ActivationFunctionType.Sqrt)
                rsd = bpool.tile([128, 1], f32, tag=f"rsd{suffix}")
                nc.vector.reciprocal(rsd[:], sd[:])
                scale = bpool.tile([128, 1], f32, tag=f"scale{suffix}")
                nc.vector.tensor_tensor(out=scale[:], in0=rsd[:], in1=gv[:],
                                        op=mybir.AluOpType.mult)
                nmean = bpool.tile([128, 1], f32, tag=f"nm{suffix}")
                nc.vector.tensor_tensor(out=nmean[:], in0=mean[:], in1=scale[:],
                                        op=mybir.AluOpType.mult)
                shift = bpool.tile([128, 1], f32, tag=f"shift{suffix}")
                nc.vector.tensor_tensor(out=shift[:], in0=btv[:], in1=nmean[:],
                                        op=mybir.AluOpType.subtract)
                return scale, shift

            sc1, sh1 = bn_layer(rst, vec["g1"], vec["bt1"], "1")
            xbn = bpool.tile([C, NSH], f32)
            nc.scalar.activation(xbn[:], rst[:],
                                 mybir.ActivationFunctionType.Identity,
                                 bias=sh1[:], scale=sc1[:])
            xbn_bf = bpool.tile([C, NSH], bft)
            nc.vector.tensor_copy(xbn_bf[:], xbn[:])

            y = bpool.tile([C, NSH], f32)
            for ci in range(NCH):
                s0_ = ci * CH
                rhs2 = xbn_bf[:, s0_:s0_ + CH]
                h1a = p2pool.tile([128, CH], f32, tag="h1a")
                h1b = p2pool.tile([128, CH], f32, tag="h1b")
                nc.tensor.matmul(h1a[:], lhsT=w["W1Ta"][:], rhs=rhs2, start=True, stop=True)
                nc.tensor.matmul(h1b[:], lhsT=w["W1Tb"][:], rhs=rhs2, start=True, stop=True)
                r1a = epool.tile([128, CH], bft, tag="r1a")
                r1b = epool.tile([128, CH], bft, tag="r1b")
                nc.scalar.activation(r1a[:], h1a[:],
                                     mybir.ActivationFunctionType.Relu,
                                     bias=vec["b1a"][:])
                nc.scalar.activation(r1b[:], h1b[:],
                                     mybir.ActivationFunctionType.Relu,
                                     bias=vec["b1b"][:])
                h2 = p2pool.tile([128, CH], f32, tag="h2")
                nc.tensor.matmul(h2[:], lhsT=w["W2Ta"][:], rhs=r1a[:], start=True, stop=False)
                nc.tensor.matmul(h2[:], lhsT=w["W2Tb"][:], rhs=r1b[:], start=False, stop=True)
                # y = h2 + b2 + xbn
                yt = epool.tile([128, CH], f32, tag="yt")
                nc.scalar.activation(yt[:], h2[:],
                                     mybir.ActivationFunctionType.Identity,
                                     bias=vec["b2"][:])
                nc.vector.tensor_tensor(out=y[:, s0_:s0_ + CH], in0=yt[:],
                                        in1=xbn[:, s0_:s0_ + CH],
                                        op=mybir.AluOpType.add)

            # BN2 partial stats only; normalization happens host-side.
            st2 = bpool.tile([128, 2], f32, tag="st2")
            nc.vector.tensor_reduce(out=st2[:, 0:1], in_=y[:],
                                    axis=mybir.AxisListType.X,
                                    op=mybir.AluOpType.add)
            sq2 = bpool.tile([C, NSH], bft, tag="sqscratch")
            nc.scalar.activation(sq2[:], y[:],
                                 mybir.ActivationFunctionType.Square,
                                 accum_out=st2[:, 1:2])
            nc.sync.dma_start(out=dstat[:, 0:2], in_=st2[:])
            nc.sync.dma_start(out=dstat[:, 2:3], in_=sc1[:])
            yout = bpool.tile([C, NSH], bft)
            nc.vector.tensor_copy(yout[:], y[:])
            nc.sync.dma_start(out=dout[:], in_=yout[:])
            p2ctx.close()
    lower_extended_insts(nc)
    return nc


def _host_prep(q, k, v, edge_feat, src, dst, Wq, Wk, Wv, We, be, Wo,
               W1, b1, W2, b2, g1, bt1, g2, bt2):
    from concurrent.futures import ThreadPoolExecutor
    ex = ThreadPoolExecutor(max_workers=8)

    def build_qkv(m):
        lo = m * NSH
        arr = np.concatenate([
            k[lo:lo + NSH].T.astype(e3m4).view(np.uint8).ravel(),
            v[lo:lo + NSH].T.astype(e3m4).view(np.uint8).ravel(),
            q[lo:lo + NSH].T.astype(e3m4).view(np.uint8).ravel()])
        dev = None
        try:
            import jax
            dev = jax.device_put(arr, jax.devices()[m])
        except Exception:
            pass
        return arr, dev

    # q/k/v don't depend on the edge sort: start their tunnel uploads NOW
    qkv_futs = [ex.submit(build_qkv, m) for m in range(NCORE)]
    # per-edge head bias on host: [E, H] instead of shipping [E, ED]
    eb_fut = ex.submit(lambda: edge_feat @ We.T + be)  # [E, H] f32, orig order
    order = np.argsort(dst, kind="stable")
    src_s = src[order]
    dst_s = dst[order]
    eb_s = eb_fut.result()[order]

    # per-core per-group tile counts
    core_of = dst_s // NSH
    grp_all = (dst_s % NSH) // 128
    tiles = []
    meta = []
    for m in range(NCORE):
        sel = core_of == m
        grp = grp_all[sel]
        cnt = np.bincount(grp, minlength=NG)
        tgc = np.maximum((cnt + 127) // 128, 1)
        tiles.append(tuple(int(x) for x in tgc))
        meta.append((sel, grp, cnt))
    # SPMD: same program on all cores -> per-group max
    tgmax = np.max(np.array(tiles), axis=0)
    offs = np.concatenate([[0], np.cumsum(tgmax)])
    T_all = int(tgmax.sum())
    S = 128 * T_all

    sqd = np.float32(1.0 / np.sqrt(np.float32(D)))
    warrs = {
        "iota": np.broadcast_to(np.arange(128, dtype=np.float32),
                                (128, 128)).astype(bf16),
        "WqT": (Wq * (sqd / WS)).T.astype(bf16),
        "WoT": (Wo / WS).T.astype(bf16),
        "W1Ta": W1[:C].T.astype(bf16),
        "W1Tb": W1[C:].T.astype(bf16),
        "W2Ta": W2.T[:C].astype(bf16),
        "W2Tb": W2.T[C:].astype(bf16),
        "WkT": (Wk * WS).T.astype(e3m4),
        "WvT": (Wv * WS).T.astype(e3m4),
        "b1a": b1[:C, None].astype(np.float32),
        "b1b": b1[C:, None].astype(np.float32),
        "b2": b2[:, None].astype(np.float32),
        "g1": g1[:, None].astype(np.float32),
        "bt1": bt1[:, None].astype(np.float32),
        "g2": g2[:, None].astype(np.float32),
        "bt2": bt2[:, None].astype(np.float32),
    }
    wblock = np.concatenate(
        [np.ascontiguousarray(warrs[nm]).view(np.uint8).ravel()
         for nm, _, _ in _WSPEC])
    assert wblock.nbytes == WBYTES
    spec = _input_spec(T_all)

    def build_core(m):
        lo = m * NSH
        sel, grp, cnt = meta[m]
        srcm = src_s[sel]
        dstm = dst_s[sel] - lo
        ebm = eb_s[sel]
        ne = len(srcm)
        # slot index per edge: edges are grp-sorted already (dst-sorted)
        gstart = np.concatenate([[0], np.cumsum(cnt)])
        within = np.arange(ne) - gstart[grp]
        slot = offs[grp] * 128 + within

        # padded slots gather row 0 (valid+finite; their one-hot column is
        # all-zero so they contribute nothing).  -1 "ignored" indices would
        # make an all-padding chunk emit ZERO descriptors -> its completion
        # semaphore never fires -> deadlock.
        src_slot = np.zeros(S, dtype=np.int64)
        src_slot[slot] = srcm
        idx16 = (src_slot >> 1).astype(np.int16)
        # wrapped for gpsimd: index j -> (j % 16, j // 16); ship 16 rows
        idxw = np.ascontiguousarray(idx16.reshape(-1, 16).T)

        parv = np.zeros(S, dtype=np.uint8)
        parv[slot] = (srcm & 1).astype(np.uint8)
        par = np.ascontiguousarray(parv.reshape(T_all, 128).T)

        drv = np.full(S, 255, dtype=np.uint8)
        drv[slot] = (dstm - grp * 128).astype(np.uint8)
        dstrel = np.ascontiguousarray(drv.reshape(T_all, 128).T)

        ebv = np.zeros((S, H), dtype=np.float32)
        ebv[slot] = ebm
        ebl = ebv.reshape(T_all, 128, H).transpose(1, 0, 2).astype(
            e3m4).reshape(128, T_all * H)

        parts = {
            "eb8": ebl, "dstrel": dstrel, "par": par, "idxw": idxw,
            "wsh": wblock[m * WSLICE:(m + 1) * WSLICE].reshape(1, WSLICE),
        }
        blob = np.concatenate(
            [np.ascontiguousarray(parts[name]).view(np.uint8).ravel()
             for name, _, _ in spec])
        qkv_arr, qkv_dev = qkv_futs[m].result()
        im = {"aqkv": qkv_arr, "blob": blob}
        try:
            # start this shard's tunnel transfer as soon as its blob is
            # ready (device_put is async; numpy work overlaps in threads)
            import jax
            im["_dev_blob"] = jax.device_put(blob, jax.devices()[m])
            if qkv_dev is not None:
                im["_dev_aqkv"] = qkv_dev
        except Exception:
            pass
        return im

    in_maps = list(ex.map(build_core, range(NCORE)))
    ex.shutdown(wait=False)
    return in_maps, tuple(tuple(int(x) for x in tgmax) for _ in range(NCORE))


DBG = {}
RUN_KW = {}
LAST = {}
_PROG_CACHE = {}
_EXEC_CACHE = {}
_ZEROS_PENDING = {}


def _run(nc, in_maps):
    """Execute via PJRT with a cached jitted executable (the stock
    run_bass_via_pjrt rebuilds jit(shard_map(...)) every call, paying
    re-trace + executable re-load).  Donated output buffers are created on
    device by a tiny jitted zeros fn (no 10MB host->device zeros shipping).
    Falls back to run_bass_kernel_spmd."""
    try:
        import jax
        import jax.numpy as jnp
        from jax.sharding import Mesh, PartitionSpec, NamedSharding
        from jax.experimental.shard_map import shard_map
        from concourse import bass2jax as b2j

        ent = _EXEC_CACHE.get(id(nc))
        if ent is None:
            b2j.install_neuronx_cc_hook()
            assert nc.dbg_addr is None
            pname = (nc.partition_id_tensor.name
                     if nc.partition_id_tensor else None)
            in_names, out_names, out_avals, zero_meta = [], [], [], []
            for alloc in nc.m.functions[0].allocations:
                if not isinstance(alloc, mybir.MemoryLocationSet):
                    continue
                name = alloc.memorylocations[0].name
                if alloc.kind == "ExternalInput":
                    if name != pname:
                        in_names.append(name)
                elif alloc.kind == "ExternalOutput":
                    shape = tuple(alloc.tensor_shape)
                    dtype = mybir.dt.np(alloc.dtype)
                    out_names.append(name)
                    out_avals.append(jax.core.ShapedArray(shape, dtype))
                    zero_meta.append((shape, dtype))
            n_params = len(in_names)
            n_outs = len(out_names)
            all_names = tuple(in_names + out_names +
                              ([pname] if pname else []))

            def _body(*args):
                operands = list(args)
                if pname is not None:
                    operands.append(b2j.partition_id_tensor())
                outs = b2j._bass_exec_p.bind(
                    *operands, out_avals=tuple(out_avals),
                    in_names=all_names, out_names=tuple(out_names),
                    lowering_input_output_aliases=(),
                    sim_require_finite=True, sim_require_nnan=True, nc=nc)
                return tuple(outs)

            devices = jax.devices()[:NCORE]
            mesh = Mesh(np.asarray(devices), ("core",))
            donate = tuple(range(n_params, n_params + n_outs))
            sharded = jax.jit(
                shard_map(_body, mesh=mesh,
                          in_specs=(PartitionSpec("core"),) * (n_params + n_outs),
                          out_specs=(PartitionSpec("core"),) * n_outs,
                          check_rep=False),
                donate_argnums=donate, keep_unused=True)
            zsh = tuple(NamedSharding(mesh, PartitionSpec("core"))
                        for _ in zero_meta)
            zfun = jax.jit(
                lambda: tuple(jnp.zeros((NCORE * s[0], *s[1:]), d)
                              for s, d in zero_meta),
                out_shardings=zsh)
            ent = (sharded, tuple(in_names), tuple(out_names),
                   tuple(out_avals), tuple(zero_meta), zfun)
            _EXEC_CACHE[id(nc)] = ent

        sharded, in_names, out_names, out_avals, zero_meta, zfun = ent
        concat_in = None
        if all(all(f"_dev_{nm}" in m for nm in in_names) for m in in_maps):
            try:
                devs = jax.devices()[:NCORE]
                nsh = NamedSharding(Mesh(np.asarray(devs), ("core",)),
                                    PartitionSpec("core"))
                concat_in = []
                for nm in in_names:
                    gshape = (sum(m[nm].shape[0] for m in in_maps),)
                    concat_in.append(jax.make_array_from_single_device_arrays(
                        gshape, nsh, [m[f"_dev_{nm}"] for m in in_maps]))
            except Exception:
                concat_in = None
        if concat_in is None:
            concat_in = [np.concatenate([np.asarray(m[nm]) for m in in_maps])
                         for nm in in_names]
        # donated output buffers: use the pre-dispatched set if kernel()
        # created one during host prep, else create now (async dispatch)
        zeros = _ZEROS_PENDING.pop(id(nc), None) or list(zfun())
        out_arrs = sharded(*concat_in, *zeros)
        # fetch all output shards in parallel (tunnel round-trips overlap);
        # order shards by their slice start -> core index
        from concurrent.futures import ThreadPoolExecutor
        shard_data = []
        for i in range(len(out_names)):
            shards = sorted(out_arrs[i].addressable_shards,
                            key=lambda s: (s.index[0].start or 0))
            shard_data.append([s.data for s in shards])
        with ThreadPoolExecutor(8) as fex:
            fetched = [list(fex.map(np.asarray, sd)) for sd in shard_data]
        return [{nm: fetched[i][c] for i, nm in enumerate(out_names)}
                for c in range(NCORE)]
    except Exception:
        res = run_bass_kernel_spmd(nc, in_maps, core_ids=list(range(NCORE)))
        return res.results


def kernel(**inputs):
    inputs = {k: np.asarray(v) for k, v in inputs.items()}
    # zeros for the donated output buffers don't depend on the inputs:
    # pre-dispatch them for already-compiled programs so their round-trip
    # overlaps host prep + input transfers
    for key, nc_c in _PROG_CACHE.items():
        ent = _EXEC_CACHE.get(id(nc_c))
        if ent is not None and id(nc_c) not in _ZEROS_PENDING:
            _ZEROS_PENDING[id(nc_c)] = list(ent[5]())
    in_maps, tiles = _host_prep(**inputs)
    # q ships in fp8; the residual path is linear through BN1/BN2, so the
    # host corrects it exactly with dq = q - fp8(q) (the nonlinear FFN-path
    # part of the error stays, ~0.5%).  Computed here so it overlaps the
    # in-flight input transfers.
    q = inputs["q"]
    dq = q - q.astype(e3m4).astype(np.float32)
    key = tiles
    nc = _PROG_CACHE.get(key)
    if nc is None:
        nc = _build_program(tiles)
        _PROG_CACHE[key] = nc
    results = _run(nc, in_maps)
    LAST["res"] = results
    # finish BN2 during unshard: combine per-shard partial sums, apply the
    # per-channel affine while transposing each shard (threaded)
    st = np.sum([r["stats"][:, 0:2] for r in results], axis=0)  # [128, 2]
    sc1 = results[0]["stats"][:, 2]      # BN1 scale (identical on all cores)
    mean = st[:, 0] / N
    var = st[:, 1] / N - mean * mean
    scale = (inputs["g2"] / np.sqrt(var + EPS)).astype(np.float32)
    shift = (inputs["bt2"] - mean * scale).astype(np.float32)
    corr = (scale * sc1).astype(np.float32)   # d(out)/d(q) on the linear path

    from concurrent.futures import ThreadPoolExecutor

    def unshard(m):
        lo = m * NSH
        return (results[m]["out"].T.astype(np.float32) * scale[None, :]
                + shift[None, :] + corr[None, :] * dq[lo:lo + NSH])

    with ThreadPoolExecutor(8) as ex:
        parts = list(ex.map(unshard, range(NCORE)))
    return np.concatenate(parts, axis=0)
